# revision 31
# baseline (speedup 1.0000x reference)
"""Trainium2 Bass kernel for nn_AttentionBlock (GroupNorm + 8-head attention + proj).

Self-contained: kernel(**inputs) takes the full unsharded inputs
(x[2,512,64,64], gamma, beta, w_qkv, b_qkv, w_proj, b_proj) and returns the
full output [2,512,64,64], running SPMD across 8 NeuronCores via
concourse.bass_utils.run_bass_kernel_spmd.

Sharding: sequence(T)-sharded, 512 queries per core. GroupNorm partial stats
are combined with a 1KB AllReduce (a dummy collective issued at t=0 absorbs
the first-collective rank-sync barrier). QKV runs as fp8 DoubleRow matmuls
(256-deep contraction per instruction); K and V^T are AllGathered in fp8
split per batch. Attention: QK^T in plain fp8 (two heads packed on the PE
array via row-disjoint tile_position — they execute concurrently), softmax
exp split across the Scalar engine (true exp -> fp8) and the Vector engine
(Schraudolph-style fast exp: relu(score*11.54+C) converted to int8 IS the
fp8e4m3 bit pattern of exp(score-2.5); the uniform e^-2.5 factor cancels in
the softmax), and A·V as fp8 DoubleRow over st-pairs (contraction 256, half
the instruction stream). Scores are ~N(0,1) so no max-subtraction is needed;
the softmax denominator rides as a ones-column folded into V^T. Output
projection + residual per slice in bf16; host concatenates the 8 slices.
"""

import math
from contextlib import ExitStack

import numpy as np
import ml_dtypes

import concourse.bass as bass
import concourse.bacc as bacc
import concourse.tile as tile
from concourse import mybir
from concourse.bass import ds, ts

B = 2
C = 512
T = 4096
H = 8
CH = 64
G = 32
EPS = 1e-5
N_CORES = 8
TQ = T // N_CORES  # 512 queries per core
SCALE = 1.0 / math.sqrt(math.sqrt(CH))
FE = 8.0 * math.log2(math.e)      # fp8e4m3 mantissa bits scale: 11.5416
SHIFT = 2.5                       # exp(s - SHIFT); cancels in softmax
FCONST = 56.0 - SHIFT * FE        # int8 byte = relu(s*FE + FCONST)

F32 = mybir.dt.float32
BF16 = mybir.dt.bfloat16
FP8 = mybir.dt.float8e4
I8 = mybir.dt.int8
AF = mybir.ActivationFunctionType
ALU = mybir.AluOpType
DR = mybir.MatmulPerfMode.DoubleRow
RG = [list(range(N_CORES))]

KSZ = C * TQ            # fp8 k payload elements per batch (262144)
WV = 80                 # padded v row: 64 ch + 1 ones + 15 pad (16B-aligned)
VSZ = TQ * H * WV       # fp8 vT payload elements per batch (327680)

# exp-engine split: near-strict alternation — even st tiles on ACT (true
# exp, ~1.34us/tile), odd st tiles on DVE (fast-exp convert, ~1.47us/tile,
# plus the denominator chain), with two odd tiles handed to ACT to balance.
DVE_ST = frozenset(st for st in range(32) if st % 2 == 1) - {7, 23}


def build(nc: bass.Bass):
    def din(name, shape, dtype=F32):
        return nc.dram_tensor(name, list(shape), dtype, kind="ExternalInput").ap()

    xq = din("xq", [B, C, TQ])
    wqT = din("wqT", [C, C], FP8)
    wkT = din("wkT", [C, C], FP8)
    wvT = din("wvT", [C, C], FP8)
    wpT = din("wpT", [H, CH, C], BF16)
    bq = din("bq", [4, 128])
    bk = din("bk", [4, 128])
    bv = din("bv", [C])
    bp = din("bp", [4, 128])
    gam = din("gam", [4, 128])
    bet = din("bet", [4, 128])
    indpair = din("indpair", [128, 64])
    indred = din("indred", [64, 8, 128])

    out = nc.dram_tensor("out", [B, C, TQ], F32, kind="ExternalOutput").ap()

    xq_stats = xq.rearrange("b (g h2 u) t -> (b g h2) (u t)", g=G, h2=2, u=8)
    xq_ct = xq.rearrange("b (ct p) t -> b ct p t", p=128)
    out_ct = out.rearrange("b (ct p) t -> b ct p t", p=128)

    with ExitStack() as octx:
        tc = octx.enter_context(tile.TileContext(nc))

        consts = octx.enter_context(tc.tile_pool(name="consts", bufs=1))
        big = octx.enter_context(tc.tile_pool(name="big", bufs=1))
        dram = octx.enter_context(tc.tile_pool(name="dram", bufs=1, space="DRAM"))

        # ---------------- constant tiles (DMAs emitted after AR issue) ----
        wq_sb = consts.tile([128, 4, C], FP8)
        wk_sb = consts.tile([128, 4, C], FP8)
        wv_sb = consts.tile([128, 4, C], FP8)
        bq_sb = consts.tile([128, 4], F32)
        bk_sb = consts.tile([128, 4], F32)
        bp_sb = consts.tile([128, 4], F32)
        gam_sb = consts.tile([128, 4], F32)
        bet_sb = consts.tile([128, 4], F32)
        bv_bc = consts.tile([128, C], F32)
        eps64 = consts.tile([64, 1], F32)
        nc.vector.memset(eps64, EPS)
        bsh = consts.tile([128, 1], F32)
        nc.vector.memset(bsh, -SHIFT)
        s12 = consts.tile([128, 8, 2], F32)
        # magic-reciprocal constant row: 1/x ~= bitcast(M - bits(x)) with
        # ~5% max error, which the softmax denominator tolerates; int32
        # subtract via tensor_tensor (tensor dtypes select the integer ALU)
        mg_i32 = consts.tile([1, 512], mybir.dt.int32)
        nc.vector.memset(mg_i32, 0x7EF31000)

        def load_consts():
            for w_sb, w_dram in ((wq_sb, wqT), (wk_sb, wkT), (wv_sb, wvT)):
                nc.sync.dma_start(w_sb[:], w_dram.rearrange("(ci p) o -> p ci o", p=128))
            for t_sb, t_dram in ((bq_sb, bq), (bk_sb, bk), (bp_sb, bp),
                                 (gam_sb, gam), (bet_sb, bet)):
                nc.sync.dma_start(t_sb[:], t_dram.rearrange("a p -> p a"))
            nc.sync.dma_start(bv_bc[:], bass.AP(tensor=bv.tensor, offset=bv.offset,
                                                ap=[[0, 128]] + list(bv.ap)))

        # ---------------- persistent big tensors ----------------
        k_sb = big.tile([128, 4, B, T], FP8)
        vT_sb = big.tile([128, B, 16, 2, H, WV], FP8)   # [t%128, b, stpair, q2, h, w]
        q_sb = big.tile([128, 4, B, TQ], FP8)
        a2_sb = big.tile([128, 4, B, TQ], BF16)      # [hp*128+p cin, hp, b, t]

        ag_in = [dram.tile([KSZ + VSZ], FP8, tag=f"ag_in{b}", name=f"ag_in{b}")
                 for b in range(B)]
        ag_out = [dram.tile([N_CORES, KSZ + VSZ], FP8, tag=f"ag_out{b}",
                            name=f"ag_out{b}", addr_space="Shared") for b in range(B)]
        ar_in = dram.tile([128, 2], F32, tag="ar_in")
        ar_out = dram.tile([128, 2], F32, tag="ar_out", addr_space="Shared")

        # =================================================================
        # Phase 1: GroupNorm statistics (local partials + AllReduce)
        # =================================================================
        with ExitStack() as ctx:
            stream = ctx.enter_context(tc.tile_pool(name="stream1", bufs=2))
            small = ctx.enter_context(tc.tile_pool(name="small", bufs=2))
            pp = ctx.enter_context(tc.tile_pool(name="pp", bufs=2, space="PSUM"))

            stats_all = small.tile([128, 8, 6], F32, tag="stats")
            for ch in range(4):
                xs = stream.tile([128, 2, 512], F32, tag="xs")
                nc.sync.dma_start(xs[:], xq_stats[:, ds(ch * 1024, 1024)]
                                  .rearrange("p (n e) -> p n e", e=512))
                for k in range(2):
                    nc.vector.bn_stats(out=stats_all[:, ch * 2 + k, :], in_=xs[:, k, :])
            mv = small.tile([128, 2], F32, tag="mv")
            nc.vector.bn_aggr(out=mv[:], in_=stats_all[:])
            vals = small.tile([128, 2], F32, tag="vals")
            nc.vector.tensor_scalar_mul(vals[:, 0:1], mv[:, 0:1], 1.0 / N_CORES)
            nc.vector.tensor_tensor(vals[:, 1:2], mv[:, 0:1], mv[:, 0:1], ALU.mult)
            nc.vector.tensor_add(vals[:, 1:2], vals[:, 1:2], mv[:, 1:2])
            nc.vector.tensor_scalar_mul(vals[:, 1:2], vals[:, 1:2], 1.0 / N_CORES)
            nc.sync.dma_start(ar_in[:], vals[:])
            nc.gpsimd.collective_compute(
                "AllReduce", ALU.add, replica_groups=RG,
                ins=[ar_in[:].opt()], outs=[ar_out[:].opt()])
            # constants + indicator DMAs land while the AllReduce is in flight
            load_consts()
            ip_sb = small.tile([128, 64], F32, tag="ip")
            nc.sync.dma_start(ip_sb[:], indpair[:])
            ir_sb = small.tile([64, 8, 128], F32, tag="ir")
            nc.sync.dma_start(ir_sb[:], indred[:])
            glob = small.tile([128, 2], F32, tag="glob")
            # gpsimd queue: a sync-queue load here would make every later
            # sync DMA (xt prefetches, ag_in writes) wait on the AllReduce
            nc.gpsimd.dma_start(glob[:], ar_out[:])
            gsum = pp.tile([64, 2], F32, tag="gsum")
            nc.tensor.matmul(gsum[:], ip_sb[:], glob[:], start=True, stop=True)
            gmean = small.tile([64, 1], F32, tag="gmean")
            nc.vector.tensor_scalar_mul(gmean[:], gsum[:, 0:1], 0.5)
            gvar = small.tile([64, 1], F32, tag="gvar")
            nc.vector.tensor_scalar_mul(gvar[:], gsum[:, 1:2], 0.5)
            gm2 = small.tile([64, 1], F32, tag="gm2")
            nc.vector.tensor_tensor(gm2[:], gmean[:], gmean[:], ALU.mult)
            nc.vector.tensor_tensor(gvar[:], gvar[:], gm2[:], ALU.subtract)
            nc.scalar.activation(out=gvar[:], in_=gvar[:], func=AF.Sqrt,
                                 bias=eps64[:], scale=1.0)
            nc.vector.reciprocal(out=gvar[:], in_=gvar[:])
            gv = small.tile([64, 2], F32, tag="gv")
            nc.vector.tensor_copy(gv[:, 0:1], gmean[:])
            nc.vector.tensor_copy(gv[:, 1:2], gvar[:])
            for bct in range(8):
                ct = bct % 4
                mr = pp.tile([128, 2], F32, tag="mr")
                nc.tensor.matmul(mr[:], ir_sb[:, bct, :], gv[:], start=True, stop=True)
                nc.vector.tensor_tensor(s12[:, bct, 0:1], mr[:, 1:2], gam_sb[:, ct:ct + 1], ALU.mult)
                tmp = small.tile([128, 1], F32, tag="tmp")
                nc.vector.tensor_tensor(tmp[:], mr[:, 0:1], s12[:, bct, 0:1], ALU.mult)
                nc.vector.tensor_tensor(s12[:, bct, 1:2], bet_sb[:, ct:ct + 1], tmp[:], ALU.subtract)

        # =================================================================
        # Phase 2: normalize local slice; local k/vT/q (fp8 DoubleRow qkv);
        # AllGather per batch
        # =================================================================
        ctx2 = ExitStack()
        with ctx2:
            hqpool = ctx2.enter_context(tc.tile_pool(name="hqpool", bufs=1))
            stg = ctx2.enter_context(tc.tile_pool(name="stg", bufs=4))
            pq = ctx2.enter_context(tc.tile_pool(name="pq", bufs=2, space="PSUM"))

            hq = hqpool.tile([128, 4, B, TQ], FP8, tag="hq")

            def normalize(b):
                for ci in range(4):
                    xt = stg.tile([128, 512], F32, tag="xt")
                    nc.sync.dma_start(xt[:], xq_ct[b, ci, :, :])
                    nc.vector.tensor_scalar(
                        out=hq[:, ci, b, :], in0=xt[:],
                        scalar1=s12[:, b * 4 + ci, 0:1], scalar2=s12[:, b * 4 + ci, 1:2],
                        op0=ALU.mult, op1=ALU.add)

            def kv_local(b):
                for co in range(4):
                    psk = pq.tile([128, 512], F32, tag="psk")
                    for m in range(2):
                        nc.tensor.matmul(psk[:], wk_sb[:, ds(2 * m, 2), ds(co * 128, 128)],
                                         hq[:, ds(2 * m, 2), b, :],
                                         start=(m == 0), stop=(m == 1), perf_mode=DR)
                    kst = stg.tile([128, 512], FP8, tag="kst")
                    nc.vector.tensor_scalar(
                        out=kst[:], in0=psk[:],
                        scalar1=bk_sb[:, co:co + 1], scalar2=None, op0=ALU.add)
                    eng = nc.sync if co % 2 == 0 else nc.scalar
                    eng.dma_start(
                        ag_in[b][0:KSZ].rearrange("(kc t) -> kc t", t=TQ)
                        [ds(co * 128, 128), :], kst[:])
                for tl in range(4):
                    psv = pq.tile([128, 512], F32, tag="psv")
                    for m in range(2):
                        nc.tensor.matmul(psv[:], hq[:, ds(2 * m, 2), b, ds(tl * 128, 128)],
                                         wv_sb[:, ds(2 * m, 2), :],
                                         start=(m == 0), stop=(m == 1), perf_mode=DR)
                    vst = stg.tile([128, H, WV], FP8, tag="vst")
                    nc.vector.tensor_tensor(vst[:, :, 0:64],
                                            psv[:].rearrange("p (h c) -> p h c", c=CH),
                                            bv_bc[:].rearrange("p (h c) -> p h c", c=CH),
                                            ALU.add)
                    nc.vector.memset(vst[:, :, 64:65], 1.0)
                    # pad columns 65:80 never reach a live output partition
                    eng = nc.sync if tl % 2 == 0 else nc.scalar
                    eng.dma_start(
                        ag_in[b][KSZ:KSZ + VSZ].rearrange("(t w) -> t w", w=H * WV)
                        [ds(tl * 128, 128), :], vst[:].rearrange("p h w -> p (h w)"))

            def q_local(b):
                for co in range(4):
                    psq = pq.tile([128, 512], F32, tag="psq")
                    for m in range(2):
                        nc.tensor.matmul(psq[:], wq_sb[:, ds(2 * m, 2), ds(co * 128, 128)],
                                         hq[:, ds(2 * m, 2), b, :],
                                         start=(m == 0), stop=(m == 1), perf_mode=DR)
                    nc.vector.tensor_scalar(
                        out=q_sb[:, co, b, :], in0=psq[:],
                        scalar1=bq_sb[:, co:co + 1], scalar2=None, op0=ALU.add)

            def ag(b):
                nc.gpsimd.collective_compute(
                    "AllGather", ALU.bypass, replica_groups=RG,
                    ins=[ag_in[b][:].opt()], outs=[ag_out[b][:].opt()])

            def load_k(b, co, eng, eng2=None):
                for rh in range(2):
                    e = eng if rh == 0 or eng2 is None else eng2
                    e.dma_start(
                        k_sb[:, co, b, ds(rh * 4 * 512, 4 * 512)]
                        .rearrange("p (r t) -> p r t", r=4),
                        ag_out[b][ds(rh * 4, 4), 0:KSZ]
                        .rearrange("r (kc t) -> kc r t", t=TQ)[ds(co * 128, 128), :, :])

            def load_v4(b, r, eng):
                # one rank's whole vT payload; rank r covers st-pairs 2r,2r+1
                eng.dma_start(
                    vT_sb[:, b, ds(r * 2, 2), :, :, :],
                    ag_out[b][r, KSZ:KSZ + VSZ]
                    .rearrange("(m q2 p w) -> p m q2 w", m=2, q2=2, p=128))

            def loads(b):
                # spread across DMA queues, rank-ordered so the m-loop never
                # waits; k co1-3 are only needed at j1/j2/j3. b1's k co1-3 go
                # last on sync (gpsimd must stay clear for the j-boundary
                # den broadcasts during b0's attention).
                if b == 0:
                    load_k(b, 0, nc.scalar, nc.sync)
                    for r in range(8):
                        load_v4(b, r, nc.scalar if r % 2 == 0 else nc.sync)
                    for co in range(1, 4):
                        load_k(b, co, nc.gpsimd)
                else:
                    load_k(b, 0, nc.sync)
                    for r in range(8):
                        load_v4(b, r, nc.sync)
                    for co in range(1, 4):
                        load_k(b, co, nc.sync)

            normalize(0)
            kv_local(0)
            ag(0)
            normalize(1)
            kv_local(1)
            ag(1)
            q_local(0)
            q_local(1)

        # (phase-2 pools closed; PSUM free for attention)
        with ExitStack() as ctx:
            loads(0)
            loads(1)

            # ==========================================================
            # attention per (b, head-pair); exp split ACT/DVE; fp8-DR AV
            # ==========================================================
            with ExitStack() as actx:
                psc = actx.enter_context(tc.tile_pool(name="psc", bufs=3, space="PSUM"))
                pav = actx.enter_context(tc.tile_pool(name="pav", bufs=1, space="PSUM"))
                epool = actx.enter_context(tc.tile_pool(name="epool", bufs=3))
                dpool = actx.enter_context(tc.tile_pool(name="dpool", bufs=3))
                wppool = actx.enter_context(tc.tile_pool(name="wppool", bufs=1))
                prstream = actx.enter_context(tc.tile_pool(name="prstream", bufs=2))

                wp_sb = wppool.tile([128, 4, C], BF16)   # [hp*128+cin, hp, cout]
                nc.sync.dma_start(wp_sb[:],
                                  wpT.rearrange("(hp w) c o -> (w c) hp o", w=2))

                den_dram = dram.tile([16, 512], F32, tag="den")

                def proj(b):
                    for co in range(4):
                        psp = psc.tile([128, 2, 512], F32, tag="ps")
                        for hp in range(4):
                            nc.tensor.matmul(psp[:, 0, :], wp_sb[:, hp, ds(co * 128, 128)],
                                             a2_sb[:, hp, b, :],
                                             start=(hp == 0), stop=(hp == 3))
                        xr = prstream.tile([128, 512], F32, tag="xr")
                        nc.sync.dma_start(xr[:], xq_ct[b, co, :, :])
                        ot = prstream.tile([128, 512], F32, tag="ot")
                        nc.vector.tensor_scalar(out=ot[:], in0=psp[:, 0, :],
                                                scalar1=bp_sb[:, co:co + 1],
                                                scalar2=None, op0=ALU.add)
                        nc.vector.tensor_tensor(ot[:], ot[:], xr[:], ALU.add)
                        nc.sync.dma_start(out_ct[b, co, :, :], ot[:])

                for b in range(B):
                    for j in range(4):
                        av = [pav.tile([80, 512], F32, tag=f"av{u}", name=f"av{u}")
                              for u in range(2)]

                        def emit_av(m, exm):
                            for u in range(2):
                                nc.tensor.matmul(av[u][:],
                                                 vT_sb[:, b, m, :, 2 * j + u, :],
                                                 exm[:, :, u, :],
                                                 start=(m == 0), stop=(m == 15),
                                                 perf_mode=DR)

                        # software-pipelined: AV(m-1) is emitted after QK(2m+1)
                        # so the in-order tensor queue never stalls on the exp
                        # of the current pair
                        ex_prev = None
                        for m in range(16):
                            ex = epool.tile([128, 2, 2, 512], FP8, tag="ex")
                            for q2 in range(2):
                                st = 2 * m + q2
                                ps = psc.tile([128, 2, 512], F32, tag="ps")
                                for u in range(2):
                                    nc.tensor.matmul(
                                        ps[:, u, :],
                                        k_sb[64 * u:64 * u + 64, j, b, ds(st * 128, 128)],
                                        q_sb[64 * u:64 * u + 64, j, b, :],
                                        start=True, stop=True, tile_position=(64 * u, 0))
                                if st in DVE_ST:
                                    nc.vector.tensor_scalar(
                                        out=ex[:, q2, :, :].bitcast(I8), in0=ps[:],
                                        scalar1=FCONST, scalar2=0.0,
                                        op0=ALU.add, op1=ALU.max)
                                else:
                                    nc.scalar.activation(
                                        out=ex[:, q2, :, :], in_=ps[:],
                                        func=AF.Exp, scale=1.0 / FE, bias=bsh[:])
                            if m > 0:
                                emit_av(m - 1, ex_prev)
                            ex_prev = ex
                        emit_av(15, ex_prev)
                        rcps = []
                        for u in range(2):
                            bh = (b * 8 + 2 * j + u) % 16
                            den_r = dpool.tile([1, 512], F32, tag="den_r")
                            nc.vector.tensor_tensor(
                                den_r[:].bitcast(mybir.dt.int32), mg_i32[:],
                                av[u][64:65, :].bitcast(mybir.dt.int32),
                                ALU.subtract)
                            nc.gpsimd.dma_start(den_dram[bh, :], den_r[:])
                            rcp_bc = dpool.tile([64, 512], F32, tag="rcp_bc")
                            rslice = den_dram[bh, :]
                            nc.gpsimd.dma_start(rcp_bc[:], bass.AP(
                                tensor=rslice.tensor, offset=rslice.offset,
                                ap=[[0, 64]] + list(rslice.ap)))
                            rcps.append(rcp_bc)
                        for u in range(2):
                            if u == 0:
                                nc.vector.tensor_tensor(a2_sb[0:64, j, b, :],
                                                        av[u][0:64, :], rcps[u][:], ALU.mult)
                            else:
                                # odd head lives on partitions 64-127: normalize
                                # into a staging tile, partition-shift via DMA
                                an = dpool.tile([64, 512], BF16, tag="an")
                                nc.vector.tensor_tensor(an[:], av[u][0:64, :],
                                                        rcps[u][:], ALU.mult)
                                nc.gpsimd.dma_start(a2_sb[64:128, j, b, :], an[:])
                        if b == 1 and j == 0:
                            # proj(0) interleaves into b1's attention stream
                            # instead of stalling the tensor engine at the
                            # batch boundary
                            proj(0)
                    if b == 1:
                        proj(1)

    return nc


def make_host_consts():
    indpair = np.zeros((128, 64), np.float32)
    for p in range(128):
        indpair[p, p // 2] = 1.0
    indred = np.zeros((64, 8, 128), np.float32)
    for bb in range(2):
        for ct in range(4):
            for p in range(128):
                row = bb * 32 + (ct * 128 + p) // 16
                indred[row, bb * 4 + ct, p] = 1.0
    return indpair, indred


def make_in_maps(x, gamma, beta, w_qkv, b_qkv, w_proj, b_proj):
    x = np.asarray(x, np.float32)
    xf = np.ascontiguousarray(x.reshape(B, C, T))
    w_qkv = np.asarray(w_qkv, np.float32)
    b_qkv = np.asarray(b_qkv, np.float32)
    w_proj = np.asarray(w_proj, np.float32)

    def f8(a):
        return np.ascontiguousarray(a).astype(ml_dtypes.float8_e4m3)

    def bf(a):
        return np.ascontiguousarray(a).astype(ml_dtypes.bfloat16)

    q_idx = np.array([h * 3 * CH + c for h in range(H) for c in range(CH)])
    k_idx = q_idx + CH
    v_idx = q_idx + 2 * CH

    wqT = f8(w_qkv[q_idx].T * (SCALE * FE))
    wkT = f8(w_qkv[k_idx].T * SCALE)
    wvT = f8(w_qkv[v_idx].T)
    wpT = bf(np.ascontiguousarray(w_proj.T).reshape(H, CH, C))
    bq = np.ascontiguousarray((b_qkv[q_idx] * (SCALE * FE)).reshape(4, 128).astype(np.float32))
    bk = np.ascontiguousarray((b_qkv[k_idx] * SCALE).reshape(4, 128).astype(np.float32))
    bv = np.ascontiguousarray(b_qkv[v_idx]).astype(np.float32)
    bp = np.ascontiguousarray(np.asarray(b_proj, np.float32).reshape(4, 128))
    gam = np.ascontiguousarray(np.asarray(gamma, np.float32).reshape(4, 128))
    bet = np.ascontiguousarray(np.asarray(beta, np.float32).reshape(4, 128))
    indpair, indred = make_host_consts()
    common = dict(wqT=wqT, wkT=wkT, wvT=wvT, wpT=wpT, bq=bq, bk=bk,
                  bv=bv, bp=bp, gam=gam, bet=bet, indpair=indpair, indred=indred)
    in_maps = []
    for i in range(N_CORES):
        m = dict(common)
        m["xq"] = np.ascontiguousarray(xf[:, :, i * TQ:(i + 1) * TQ])
        in_maps.append(m)
    return in_maps


def assemble_output(results):
    parts = [results[i]["out"] for i in range(N_CORES)]
    full = np.concatenate(parts, axis=2)  # [B, C, T]
    return full.reshape(B, C, 64, 64)


# ---------------------------------------------------------------------------
# public entry point
# ---------------------------------------------------------------------------
_compiled_nc = None


def _get_nc():
    global _compiled_nc
    if _compiled_nc is None:
        nc = bacc.Bacc("TRN2", target_bir_lowering=False, debug=False,
                       num_devices=N_CORES)
        build(nc)
        nc.compile()
        _compiled_nc = nc
    return _compiled_nc


def run(inputs, trace=False):
    """Compile (cached), run SPMD on cores 0-7, return (full_output, results)."""
    from concourse import bass_utils
    nc = _get_nc()
    in_maps = make_in_maps(**inputs)
    res = bass_utils.run_bass_kernel_spmd(
        nc, in_maps, core_ids=list(range(N_CORES)), trace=trace)
    out = assemble_output(res.results).astype(np.float32)
    return out, res


def kernel(x, gamma, beta, w_qkv, b_qkv, w_proj, b_proj):
    out, _ = run(dict(x=x, gamma=gamma, beta=beta, w_qkv=w_qkv, b_qkv=b_qkv,
                      w_proj=w_proj, b_proj=b_proj))
    return out


# revision 33
# speedup vs baseline: 1.0687x; 1.0687x over previous
"""Trainium2 Bass kernel for nn_AttentionBlock (GroupNorm + 8-head attention + proj).

Self-contained: kernel(**inputs) takes the full unsharded inputs
(x[2,512,64,64], gamma, beta, w_qkv, b_qkv, w_proj, b_proj) and returns the
full output [2,512,64,64], running SPMD across 8 NeuronCores via
concourse.bass_utils.run_bass_kernel_spmd.

Sharding: sequence(T)-sharded, 512 queries per core. GroupNorm partial stats
are combined with a 1KB AllReduce (a dummy collective issued at t=0 absorbs
the first-collective rank-sync barrier). QKV runs as fp8 DoubleRow matmuls
(256-deep contraction per instruction); K and V^T are AllGathered in fp8
split per batch. Attention: QK^T in plain fp8 (two heads packed on the PE
array via row-disjoint tile_position — they execute concurrently), softmax
exp split across the Scalar engine (true exp -> fp8) and the Vector engine
(Schraudolph-style fast exp: relu(score*11.54+C) converted to int8 IS the
fp8e4m3 bit pattern of exp(score-2.5); the uniform e^-2.5 factor cancels in
the softmax), and A·V as fp8 DoubleRow over st-pairs (contraction 256, half
the instruction stream). Scores are ~N(0,1) so no max-subtraction is needed;
the softmax denominator rides as a ones-column folded into V^T. Output
projection + residual per slice in bf16; host concatenates the 8 slices.
"""

import math
from contextlib import ExitStack

import numpy as np
import ml_dtypes

import concourse.bass as bass
import concourse.bacc as bacc
import concourse.tile as tile
from concourse import mybir
from concourse.bass import ds, ts

B = 2
C = 512
T = 4096
H = 8
CH = 64
G = 32
EPS = 1e-5
N_CORES = 8
TQ = T // N_CORES  # 512 queries per core
SCALE = 1.0 / math.sqrt(math.sqrt(CH))
FE = 8.0 * math.log2(math.e)      # fp8e4m3 mantissa bits scale: 11.5416
SHIFT = 2.5                       # exp(s - SHIFT); cancels in softmax
FCONST = 56.0 - SHIFT * FE        # int8 byte = relu(s*FE + FCONST)

F32 = mybir.dt.float32
BF16 = mybir.dt.bfloat16
FP8 = mybir.dt.float8e4
I8 = mybir.dt.int8
AF = mybir.ActivationFunctionType
ALU = mybir.AluOpType
DR = mybir.MatmulPerfMode.DoubleRow
RG = [list(range(N_CORES))]

KSZ = C * TQ            # fp8 k payload elements per batch (262144)
WV = 80                 # padded v row: 64 ch + 1 ones + 15 pad (16B-aligned)
VSZ = TQ * H * WV       # fp8 vT payload elements per batch (327680)

# exp-engine split: near-strict alternation — even st tiles on ACT (true
# exp, ~1.34us/tile), odd st tiles on DVE (fast-exp convert, ~1.47us/tile,
# plus the denominator chain), with two odd tiles handed to ACT to balance.
DVE_ST = frozenset(st for st in range(32) if st % 2 == 1) - {7, 23}


def build(nc: bass.Bass):
    def din(name, shape, dtype=F32):
        return nc.dram_tensor(name, list(shape), dtype, kind="ExternalInput").ap()

    xq = din("xq", [B, C, TQ])
    wqT = din("wqT", [C, C], FP8)
    wkT = din("wkT", [C, C], FP8)
    wvT = din("wvT", [C, C], FP8)
    wpT = din("wpT", [H, CH, C], BF16)
    bq = din("bq", [4, 128])
    bk = din("bk", [4, 128])
    bv = din("bv", [C])
    bp = din("bp", [4, 128])
    gam = din("gam", [4, 128])
    bet = din("bet", [4, 128])
    indpair = din("indpair", [128, 64])
    indred = din("indred", [64, 8, 128])

    out = nc.dram_tensor("out", [B, C, TQ], F32, kind="ExternalOutput").ap()

    xq_stats = xq.rearrange("b (g h2 u) t -> (b g h2) (u t)", g=G, h2=2, u=8)
    xq_ct = xq.rearrange("b (ct p) t -> b ct p t", p=128)
    out_ct = out.rearrange("b (ct p) t -> b ct p t", p=128)

    with ExitStack() as octx:
        tc = octx.enter_context(tile.TileContext(nc))

        consts = octx.enter_context(tc.tile_pool(name="consts", bufs=1))
        big = octx.enter_context(tc.tile_pool(name="big", bufs=1))
        dram = octx.enter_context(tc.tile_pool(name="dram", bufs=1, space="DRAM"))

        # ---------------- constant tiles (DMAs emitted after AR issue) ----
        wq_sb = consts.tile([128, 4, C], FP8)
        wk_sb = consts.tile([128, 4, C], FP8)
        wv_sb = consts.tile([128, 4, C], FP8)
        bq_sb = consts.tile([128, 4], F32)
        bk_sb = consts.tile([128, 4], F32)
        bp_sb = consts.tile([128, 4], F32)
        gam_sb = consts.tile([128, 4], F32)
        bet_sb = consts.tile([128, 4], F32)
        bv_bc = consts.tile([128, C], F32)
        eps64 = consts.tile([64, 1], F32)
        nc.vector.memset(eps64, EPS)
        bsh = consts.tile([128, 1], F32)
        nc.vector.memset(bsh, -SHIFT)
        s12 = consts.tile([128, 8, 2], F32)
        # magic-reciprocal constant row: 1/x ~= bitcast(M - bits(x)) with
        # ~5% max error, which the softmax denominator tolerates; int32
        # subtract via tensor_tensor (tensor dtypes select the integer ALU)
        mg_i32 = consts.tile([1, 512], mybir.dt.int32)
        nc.vector.memset(mg_i32, 0x7EF31000)
        # warm up the gpsimd partition_broadcast DSP library at t~0 so the
        # ~16us lib load doesn't land on the first attention j-boundary
        warm_bc = consts.tile([64, 4], F32)
        nc.gpsimd.partition_broadcast(warm_bc[:], mg_i32[:, 0:4].bitcast(F32))

        def load_consts():
            for w_sb, w_dram in ((wq_sb, wqT), (wk_sb, wkT), (wv_sb, wvT)):
                nc.sync.dma_start(w_sb[:], w_dram.rearrange("(ci p) o -> p ci o", p=128))
            for t_sb, t_dram in ((bq_sb, bq), (bk_sb, bk), (bp_sb, bp),
                                 (gam_sb, gam), (bet_sb, bet)):
                nc.sync.dma_start(t_sb[:], t_dram.rearrange("a p -> p a"))
            nc.sync.dma_start(bv_bc[:], bass.AP(tensor=bv.tensor, offset=bv.offset,
                                                ap=[[0, 128]] + list(bv.ap)))

        # ---------------- persistent big tensors ----------------
        k_sb = big.tile([128, 4, B, T], FP8)
        vT_sb = big.tile([128, B, 16, 2, H, WV], FP8)   # [t%128, b, stpair, q2, h, w]
        q_sb = big.tile([128, 4, B, TQ], FP8)
        a2_sb = big.tile([128, 4, B, TQ], BF16)      # [hp*128+p cin, hp, b, t]

        ag_in = [dram.tile([KSZ + VSZ], FP8, tag=f"ag_in{b}", name=f"ag_in{b}")
                 for b in range(B)]
        ag_out = [dram.tile([N_CORES, KSZ + VSZ], FP8, tag=f"ag_out{b}",
                            name=f"ag_out{b}", addr_space="Shared") for b in range(B)]
        ar_in = dram.tile([128, 2], F32, tag="ar_in")
        ar_out = dram.tile([128, 2], F32, tag="ar_out", addr_space="Shared")

        # =================================================================
        # Phase 1: GroupNorm statistics (local partials + AllReduce)
        # =================================================================
        with ExitStack() as ctx:
            stream = ctx.enter_context(tc.tile_pool(name="stream1", bufs=2))
            small = ctx.enter_context(tc.tile_pool(name="small", bufs=2))
            pp = ctx.enter_context(tc.tile_pool(name="pp", bufs=2, space="PSUM"))

            stats_all = small.tile([128, 8, 6], F32, tag="stats")
            for ch in range(4):
                xs = stream.tile([128, 2, 512], F32, tag="xs")
                nc.sync.dma_start(xs[:], xq_stats[:, ds(ch * 1024, 1024)]
                                  .rearrange("p (n e) -> p n e", e=512))
                for k in range(2):
                    nc.vector.bn_stats(out=stats_all[:, ch * 2 + k, :], in_=xs[:, k, :])
            mv = small.tile([128, 2], F32, tag="mv")
            nc.vector.bn_aggr(out=mv[:], in_=stats_all[:])
            vals = small.tile([128, 2], F32, tag="vals")
            nc.vector.tensor_scalar_mul(vals[:, 0:1], mv[:, 0:1], 1.0 / N_CORES)
            nc.vector.tensor_tensor(vals[:, 1:2], mv[:, 0:1], mv[:, 0:1], ALU.mult)
            nc.vector.tensor_add(vals[:, 1:2], vals[:, 1:2], mv[:, 1:2])
            nc.vector.tensor_scalar_mul(vals[:, 1:2], vals[:, 1:2], 1.0 / N_CORES)
            nc.sync.dma_start(ar_in[:], vals[:])
            nc.gpsimd.collective_compute(
                "AllReduce", ALU.add, replica_groups=RG,
                ins=[ar_in[:].opt()], outs=[ar_out[:].opt()])
            # constants + indicator DMAs land while the AllReduce is in flight
            load_consts()
            ip_sb = small.tile([128, 64], F32, tag="ip")
            nc.sync.dma_start(ip_sb[:], indpair[:])
            ir_sb = small.tile([64, 8, 128], F32, tag="ir")
            nc.sync.dma_start(ir_sb[:], indred[:])
            glob = small.tile([128, 2], F32, tag="glob")
            # gpsimd queue: a sync-queue load here would make every later
            # sync DMA (xt prefetches, ag_in writes) wait on the AllReduce
            nc.gpsimd.dma_start(glob[:], ar_out[:])
            gsum = pp.tile([64, 2], F32, tag="gsum")
            nc.tensor.matmul(gsum[:], ip_sb[:], glob[:], start=True, stop=True)
            gmean = small.tile([64, 1], F32, tag="gmean")
            nc.vector.tensor_scalar_mul(gmean[:], gsum[:, 0:1], 0.5)
            gvar = small.tile([64, 1], F32, tag="gvar")
            nc.vector.tensor_scalar_mul(gvar[:], gsum[:, 1:2], 0.5)
            gm2 = small.tile([64, 1], F32, tag="gm2")
            nc.vector.tensor_tensor(gm2[:], gmean[:], gmean[:], ALU.mult)
            nc.vector.tensor_tensor(gvar[:], gvar[:], gm2[:], ALU.subtract)
            nc.scalar.activation(out=gvar[:], in_=gvar[:], func=AF.Sqrt,
                                 bias=eps64[:], scale=1.0)
            nc.vector.reciprocal(out=gvar[:], in_=gvar[:])
            gv = small.tile([64, 2], F32, tag="gv")
            nc.vector.tensor_copy(gv[:, 0:1], gmean[:])
            nc.vector.tensor_copy(gv[:, 1:2], gvar[:])
            for bct in range(8):
                ct = bct % 4
                mr = pp.tile([128, 2], F32, tag="mr")
                nc.tensor.matmul(mr[:], ir_sb[:, bct, :], gv[:], start=True, stop=True)
                nc.vector.tensor_tensor(s12[:, bct, 0:1], mr[:, 1:2], gam_sb[:, ct:ct + 1], ALU.mult)
                tmp = small.tile([128, 1], F32, tag="tmp")
                nc.vector.tensor_tensor(tmp[:], mr[:, 0:1], s12[:, bct, 0:1], ALU.mult)
                nc.vector.tensor_tensor(s12[:, bct, 1:2], bet_sb[:, ct:ct + 1], tmp[:], ALU.subtract)

        # =================================================================
        # Phase 2: normalize local slice; local k/vT/q (fp8 DoubleRow qkv);
        # AllGather per batch
        # =================================================================
        ctx2 = ExitStack()
        with ctx2:
            hqpool = ctx2.enter_context(tc.tile_pool(name="hqpool", bufs=1))
            stg = ctx2.enter_context(tc.tile_pool(name="stg", bufs=4))
            pq = ctx2.enter_context(tc.tile_pool(name="pq", bufs=2, space="PSUM"))

            hq = hqpool.tile([128, 4, B, TQ], FP8, tag="hq")

            def normalize(b):
                for ci in range(4):
                    xt = stg.tile([128, 512], F32, tag="xt")
                    nc.sync.dma_start(xt[:], xq_ct[b, ci, :, :])
                    nc.vector.tensor_scalar(
                        out=hq[:, ci, b, :], in0=xt[:],
                        scalar1=s12[:, b * 4 + ci, 0:1], scalar2=s12[:, b * 4 + ci, 1:2],
                        op0=ALU.mult, op1=ALU.add)

            def kv_local(b):
                for co in range(4):
                    psk = pq.tile([128, 512], F32, tag="psk")
                    for m in range(2):
                        nc.tensor.matmul(psk[:], wk_sb[:, ds(2 * m, 2), ds(co * 128, 128)],
                                         hq[:, ds(2 * m, 2), b, :],
                                         start=(m == 0), stop=(m == 1), perf_mode=DR)
                    kst = stg.tile([128, 512], FP8, tag="kst")
                    nc.vector.tensor_scalar(
                        out=kst[:], in0=psk[:],
                        scalar1=bk_sb[:, co:co + 1], scalar2=None, op0=ALU.add)
                    eng = nc.sync if co % 2 == 0 else nc.scalar
                    eng.dma_start(
                        ag_in[b][0:KSZ].rearrange("(kc t) -> kc t", t=TQ)
                        [ds(co * 128, 128), :], kst[:])
                for tl in range(4):
                    psv = pq.tile([128, 512], F32, tag="psv")
                    for m in range(2):
                        nc.tensor.matmul(psv[:], hq[:, ds(2 * m, 2), b, ds(tl * 128, 128)],
                                         wv_sb[:, ds(2 * m, 2), :],
                                         start=(m == 0), stop=(m == 1), perf_mode=DR)
                    vst = stg.tile([128, H, WV], FP8, tag="vst")
                    nc.vector.tensor_tensor(vst[:, :, 0:64],
                                            psv[:].rearrange("p (h c) -> p h c", c=CH),
                                            bv_bc[:].rearrange("p (h c) -> p h c", c=CH),
                                            ALU.add)
                    nc.vector.memset(vst[:, :, 64:65], 1.0)
                    # pad columns 65:80 never reach a live output partition
                    eng = nc.sync if tl % 2 == 0 else nc.scalar
                    eng.dma_start(
                        ag_in[b][KSZ:KSZ + VSZ].rearrange("(t w) -> t w", w=H * WV)
                        [ds(tl * 128, 128), :], vst[:].rearrange("p h w -> p (h w)"))

            def q_local(b):
                for co in range(4):
                    psq = pq.tile([128, 512], F32, tag="psq")
                    for m in range(2):
                        nc.tensor.matmul(psq[:], wq_sb[:, ds(2 * m, 2), ds(co * 128, 128)],
                                         hq[:, ds(2 * m, 2), b, :],
                                         start=(m == 0), stop=(m == 1), perf_mode=DR)
                    nc.vector.tensor_scalar(
                        out=q_sb[:, co, b, :], in0=psq[:],
                        scalar1=bq_sb[:, co:co + 1], scalar2=None, op0=ALU.add)

            def ag(b):
                nc.gpsimd.collective_compute(
                    "AllGather", ALU.bypass, replica_groups=RG,
                    ins=[ag_in[b][:].opt()], outs=[ag_out[b][:].opt()])

            def load_k(b, co, eng, eng2=None):
                for rh in range(2):
                    e = eng if rh == 0 or eng2 is None else eng2
                    e.dma_start(
                        k_sb[:, co, b, ds(rh * 4 * 512, 4 * 512)]
                        .rearrange("p (r t) -> p r t", r=4),
                        ag_out[b][ds(rh * 4, 4), 0:KSZ]
                        .rearrange("r (kc t) -> kc r t", t=TQ)[ds(co * 128, 128), :, :])

            def load_v4(b, r, eng):
                # one rank's whole vT payload; rank r covers st-pairs 2r,2r+1
                eng.dma_start(
                    vT_sb[:, b, ds(r * 2, 2), :, :, :],
                    ag_out[b][r, KSZ:KSZ + VSZ]
                    .rearrange("(m q2 p w) -> p m q2 w", m=2, q2=2, p=128))

            def loads(b):
                # spread across DMA queues, rank-ordered so the m-loop never
                # waits; k co1-3 are only needed at j1/j2/j3. b1's k co1-3 go
                # last on sync (gpsimd must stay clear for the j-boundary
                # den broadcasts during b0's attention).
                if b == 0:
                    load_k(b, 0, nc.scalar, nc.sync)
                    for r in range(8):
                        load_v4(b, r, nc.scalar if r % 2 == 0 else nc.sync)
                    for co in range(1, 4):
                        load_k(b, co, nc.gpsimd)
                else:
                    load_k(b, 0, nc.sync)
                    for r in range(8):
                        load_v4(b, r, nc.sync)
                    for co in range(1, 4):
                        load_k(b, co, nc.sync)

            normalize(0)
            kv_local(0)
            ag(0)
            normalize(1)
            kv_local(1)
            ag(1)
            q_local(0)
            q_local(1)

        # (phase-2 pools closed; PSUM free for attention)
        with ExitStack() as ctx:
            loads(0)
            loads(1)

            # ==========================================================
            # attention per (b, head-pair); exp split ACT/DVE; fp8-DR AV
            # ==========================================================
            with ExitStack() as actx:
                psc = actx.enter_context(tc.tile_pool(name="psc", bufs=3, space="PSUM"))
                pav = actx.enter_context(tc.tile_pool(name="pav", bufs=1, space="PSUM"))
                epool = actx.enter_context(tc.tile_pool(name="epool", bufs=3))
                dpool = actx.enter_context(tc.tile_pool(name="dpool", bufs=3))
                wppool = actx.enter_context(tc.tile_pool(name="wppool", bufs=1))
                prstream = actx.enter_context(tc.tile_pool(name="prstream", bufs=2))

                wp_sb = wppool.tile([128, 4, C], BF16)   # [hp*128+cin, hp, cout]
                nc.sync.dma_start(wp_sb[:],
                                  wpT.rearrange("(hp w) c o -> (w c) hp o", w=2))

                den_dram = dram.tile([16, 512], F32, tag="den")

                def proj(b):
                    for co in range(4):
                        psp = psc.tile([128, 2, 512], F32, tag="ps")
                        for hp in range(4):
                            nc.tensor.matmul(psp[:, 0, :], wp_sb[:, hp, ds(co * 128, 128)],
                                             a2_sb[:, hp, b, :],
                                             start=(hp == 0), stop=(hp == 3))
                        xr = prstream.tile([128, 512], F32, tag="xr")
                        nc.sync.dma_start(xr[:], xq_ct[b, co, :, :])
                        ot = prstream.tile([128, 512], F32, tag="ot")
                        nc.vector.tensor_scalar(out=ot[:], in0=psp[:, 0, :],
                                                scalar1=bp_sb[:, co:co + 1],
                                                scalar2=None, op0=ALU.add)
                        nc.vector.tensor_tensor(ot[:], ot[:], xr[:], ALU.add)
                        nc.sync.dma_start(out_ct[b, co, :, :], ot[:])

                for b in range(B):
                    for j in range(4):
                        av = [pav.tile([80, 512], F32, tag=f"av{u}", name=f"av{u}")
                              for u in range(2)]

                        def emit_av(m, exm):
                            for u in range(2):
                                nc.tensor.matmul(av[u][:],
                                                 vT_sb[:, b, m, :, 2 * j + u, :],
                                                 exm[:, :, u, :],
                                                 start=(m == 0), stop=(m == 15),
                                                 perf_mode=DR)

                        # software-pipelined: AV(m-1) is emitted after QK(2m+1)
                        # so the in-order tensor queue never stalls on the exp
                        # of the current pair
                        ex_prev = None
                        for m in range(16):
                            ex = epool.tile([128, 2, 2, 512], FP8, tag="ex")
                            for q2 in range(2):
                                st = 2 * m + q2
                                ps = psc.tile([128, 2, 512], F32, tag="ps")
                                for u in range(2):
                                    nc.tensor.matmul(
                                        ps[:, u, :],
                                        k_sb[64 * u:64 * u + 64, j, b, ds(st * 128, 128)],
                                        q_sb[64 * u:64 * u + 64, j, b, :],
                                        start=True, stop=True, tile_position=(64 * u, 0))
                                if st in DVE_ST:
                                    nc.vector.tensor_scalar(
                                        out=ex[:, q2, :, :].bitcast(I8), in0=ps[:],
                                        scalar1=FCONST, scalar2=0.0,
                                        op0=ALU.add, op1=ALU.max)
                                else:
                                    nc.scalar.activation(
                                        out=ex[:, q2, :, :], in_=ps[:],
                                        func=AF.Exp, scale=1.0 / FE, bias=bsh[:])
                            if m > 0:
                                emit_av(m - 1, ex_prev)
                            ex_prev = ex
                        emit_av(15, ex_prev)
                        rcps = []
                        for u in range(2):
                            den_r = dpool.tile([1, 512], F32, tag="den_r")
                            nc.vector.tensor_tensor(
                                den_r[:].bitcast(mybir.dt.int32), mg_i32[:],
                                av[u][64:65, :].bitcast(mybir.dt.int32),
                                ALU.subtract)
                            rcp_bc = dpool.tile([64, 512], F32, tag="rcp_bc")
                            nc.gpsimd.partition_broadcast(rcp_bc[:], den_r[:])
                            rcps.append(rcp_bc)
                        for u in range(2):
                            if u == 0:
                                nc.vector.tensor_tensor(a2_sb[0:64, j, b, :],
                                                        av[u][0:64, :], rcps[u][:], ALU.mult)
                            else:
                                # odd head lives on partitions 64-127: normalize
                                # into a staging tile, partition-shift via DMA
                                an = dpool.tile([64, 512], BF16, tag="an")
                                nc.vector.tensor_tensor(an[:], av[u][0:64, :],
                                                        rcps[u][:], ALU.mult)
                                nc.gpsimd.dma_start(a2_sb[64:128, j, b, :], an[:])
                        if b == 1 and j == 0:
                            # proj(0) interleaves into b1's attention stream
                            # instead of stalling the tensor engine at the
                            # batch boundary
                            proj(0)
                    if b == 1:
                        proj(1)

    return nc


def make_host_consts():
    indpair = np.zeros((128, 64), np.float32)
    for p in range(128):
        indpair[p, p // 2] = 1.0
    indred = np.zeros((64, 8, 128), np.float32)
    for bb in range(2):
        for ct in range(4):
            for p in range(128):
                row = bb * 32 + (ct * 128 + p) // 16
                indred[row, bb * 4 + ct, p] = 1.0
    return indpair, indred


def make_in_maps(x, gamma, beta, w_qkv, b_qkv, w_proj, b_proj):
    x = np.asarray(x, np.float32)
    xf = np.ascontiguousarray(x.reshape(B, C, T))
    w_qkv = np.asarray(w_qkv, np.float32)
    b_qkv = np.asarray(b_qkv, np.float32)
    w_proj = np.asarray(w_proj, np.float32)

    def f8(a):
        return np.ascontiguousarray(a).astype(ml_dtypes.float8_e4m3)

    def bf(a):
        return np.ascontiguousarray(a).astype(ml_dtypes.bfloat16)

    q_idx = np.array([h * 3 * CH + c for h in range(H) for c in range(CH)])
    k_idx = q_idx + CH
    v_idx = q_idx + 2 * CH

    wqT = f8(w_qkv[q_idx].T * (SCALE * FE))
    wkT = f8(w_qkv[k_idx].T * SCALE)
    wvT = f8(w_qkv[v_idx].T)
    wpT = bf(np.ascontiguousarray(w_proj.T).reshape(H, CH, C))
    bq = np.ascontiguousarray((b_qkv[q_idx] * (SCALE * FE)).reshape(4, 128).astype(np.float32))
    bk = np.ascontiguousarray((b_qkv[k_idx] * SCALE).reshape(4, 128).astype(np.float32))
    bv = np.ascontiguousarray(b_qkv[v_idx]).astype(np.float32)
    bp = np.ascontiguousarray(np.asarray(b_proj, np.float32).reshape(4, 128))
    gam = np.ascontiguousarray(np.asarray(gamma, np.float32).reshape(4, 128))
    bet = np.ascontiguousarray(np.asarray(beta, np.float32).reshape(4, 128))
    indpair, indred = make_host_consts()
    common = dict(wqT=wqT, wkT=wkT, wvT=wvT, wpT=wpT, bq=bq, bk=bk,
                  bv=bv, bp=bp, gam=gam, bet=bet, indpair=indpair, indred=indred)
    in_maps = []
    for i in range(N_CORES):
        m = dict(common)
        m["xq"] = np.ascontiguousarray(xf[:, :, i * TQ:(i + 1) * TQ])
        in_maps.append(m)
    return in_maps


def assemble_output(results):
    parts = [results[i]["out"] for i in range(N_CORES)]
    full = np.concatenate(parts, axis=2)  # [B, C, T]
    return full.reshape(B, C, 64, 64)


# ---------------------------------------------------------------------------
# public entry point
# ---------------------------------------------------------------------------
_compiled_nc = None


def _get_nc():
    global _compiled_nc
    if _compiled_nc is None:
        nc = bacc.Bacc("TRN2", target_bir_lowering=False, debug=False,
                       num_devices=N_CORES)
        build(nc)
        nc.compile()
        _compiled_nc = nc
    return _compiled_nc


def run(inputs, trace=False):
    """Compile (cached), run SPMD on cores 0-7, return (full_output, results)."""
    from concourse import bass_utils
    nc = _get_nc()
    in_maps = make_in_maps(**inputs)
    res = bass_utils.run_bass_kernel_spmd(
        nc, in_maps, core_ids=list(range(N_CORES)), trace=trace)
    out = assemble_output(res.results).astype(np.float32)
    return out, res


def kernel(x, gamma, beta, w_qkv, b_qkv, w_proj, b_proj):
    out, _ = run(dict(x=x, gamma=gamma, beta=beta, w_qkv=w_qkv, b_qkv=b_qkv,
                      w_proj=w_proj, b_proj=b_proj))
    return out
